# revision 22
# baseline (speedup 1.0000x reference)
"""PatientMeanEncoder Trainium2 kernel.

Computes, for full inputs (dem [64,10], timesteps [64,2048,256], MLP weights):
    d = relu(relu(dem@w1+b1)@w2+b2)                      # [64,20]
    x = concat([timesteps, broadcast(d)], -1)            # [64,2048,276]
    out = relu(cumsum(x,1) / max(cumsum(x!=0,1), 1))     # [64,2048,276]

Sharding: pure data parallel over 8 NeuronCores, 8 patients per core
(timesteps/out sliced on N; tiny MLP weights replicated; each core runs
its own patients' MLP rows).

Core algorithm (both modes): per patient, the causal cumulative sums
live in a PSUM bank. For each 128-row L-chunk, a matmul with the
inclusive upper-triangular T (T[k,m]=1 for k<=m) accumulates the
in-chunk prefix on top of the carry already in the bank; after the
readout, a strictly-lower U' (U'[k,m]=1 for k>m) matmul tops the bank
up to the full-chunk column total, which is exactly the carry the next
chunk needs. Each element passes the PE twice; everything stays on-chip.

Two compiled variants, dispatched on the host by scanning the input:

- fast: valid when timesteps contains no exact zeros. Then the nonzero
  count for the timesteps channels is deterministically l+1, so the
  whole count cumsum disappears; the readout is a single tensor_scalar
  (relu via op0=max, then multiply by a host-precomputed per-partition
  1/(l+1) column). x is split on the host into bf16 hi+lo (x ~= hi+lo
  to ~2^-17): same DMA bytes as f32, but the matmuls run at full bf16
  PE rate instead of the ~3x-slower fp32 path, with hi and lo
  accumulating into the same PSUM columns.

- general: correct for any input. ind = (x != 0) + 1e-35 is computed on
  DVE and rides in the same [x | ind] fp32r moving operand (FD=512);
  the epsilon keeps count>0 everywhere (where the true count is 0 the
  cumsum is exactly 0, so out = 0 * huge = 0), removing any clamp op.
  Readout is a custom-DVE approximate reciprocal (~51 ULP) plus one
  scalar_tensor_tensor.

The dem block of the output is d broadcast along L (exactly d: for
those channels avg == d whether d is zero or not): a per-patient SBUF
tile written once via a DRAM-bounce broadcast DMA, copied into each
output tile by the otherwise-idle ACT engine.
"""

import os
import sys
import types
import numpy as np

# Problem constants (hardcoded per contract; kernel.py is self-contained).
N, L, C, DEM = 64, 2048, 256, 10
H1, DEMF = 40, 20
NCORES = 8
NPC = N // NCORES            # patients per core
P = 128                      # partitions = rows per L-chunk
NCHUNKS = L // P             # 16
G = 2                        # L-chunks per DMA group
COUT = C + DEMF              # 276
EPS = 1e-35

_COMPILED = {}
LAST_EXEC_NS = None
LAST_MODE = None
TRACE = os.environ.get("PME_TRACE", "1") == "1"


def _register_ntff_hook():
    """This image's antenv lacks axon_hooks; synthesize it so
    run_bass_kernel_spmd(trace=True) can capture NTFF profiles.
    Degrades silently (trace is skipped) if anything is missing."""
    try:
        import antenv.axon_hooks  # noqa: F401
        return
    except Exception:
        pass
    try:
        from trn_agent_boot.trn_boot import _ntff_profile_via_ctypes

        hook = _ntff_profile_via_ctypes("/opt/axon/libaxon_pjrt.so")
        mod = types.ModuleType("antenv.axon_hooks")
        mod.get_axon_ntff_profile_hook = lambda: hook
        mod.set_axon_ntff_profile_hook = lambda h: None
        sys.modules["antenv.axon_hooks"] = mod
        import antenv

        antenv.axon_hooks = mod
    except Exception:
        pass


def _emit_mlp_and_demb(nc, tc, mybir, npc, demT, w1a, w2a, demb):
    """dem_fc MLP (biases folded via augmented ones row/column) +
    per-patient broadcast tiles of d along the partition dim."""
    f32 = mybir.dt.float32
    AF = mybir.ActivationFunctionType
    with tc.tile_pool(name="mlps", bufs=1) as mlps, \
         tc.tile_pool(name="mlpp", bufs=2, space="PSUM") as mlpp:
        demT_t = mlps.tile([DEM + 1, npc], f32)
        nc.gpsimd.dma_start(demT_t[:], demT.ap())
        w1_t = mlps.tile([DEM + 1, H1], f32)
        nc.gpsimd.dma_start(w1_t[:], w1a.ap())
        w2_t = mlps.tile([H1 + 1, DEMF], f32)
        nc.gpsimd.dma_start(w2_t[:], w2a.ap())
        p1 = mlpp.tile([H1, npc], f32)
        nc.tensor.matmul(p1[:], w1_t[:], demT_t[:], start=True, stop=True)
        h1 = mlps.tile([H1 + 1, npc], f32)
        nc.vector.memset(h1[:], 1.0)  # row H1 stays 1.0 (bias input)
        nc.scalar.activation(h1[0:H1, :], p1[:], AF.Relu)
        p2 = mlpp.tile([npc, DEMF], f32)
        nc.tensor.matmul(p2[:], h1[:], w2_t[:], start=True, stop=True)
        d_t = mlps.tile([npc, DEMF], f32)
        nc.scalar.activation(d_t[:], p2[:], AF.Relu)
        # SBUF APs can't partition-broadcast in DMA; bounce via DRAM.
        dscratch = nc.dram_tensor("dscratch", [npc, DEMF], f32)
        nc.gpsimd.dma_start(dscratch.ap(), d_t[:])
        for pi in range(npc):
            nc.gpsimd.dma_start(
                demb[:, pi * DEMF:(pi + 1) * DEMF],
                dscratch.ap()[pi, :].partition_broadcast(P))


def _build_fast(npc=NPC, nchunks=NCHUNKS, g=4, inbufs=16, outbufs=24):
    """No-exact-zeros variant: count == l+1, x as bf16 hi+lo."""
    import concourse.mybir as mybir
    import concourse.tile as tile
    from concourse import bacc
    from contextlib import ExitStack

    f32 = mybir.dt.float32
    bf16 = mybir.dt.bfloat16
    AF = mybir.ActivationFunctionType
    OP = mybir.AluOpType

    l = nchunks * P
    ng = nchunks // g

    nc = bacc.Bacc("TRN2", target_bir_lowering=False, debug=False,
                   num_devices=NCORES)
    # host-packed [hi | lo] bf16 per row: [npc, l, 2*C] bf16
    ts = nc.dram_tensor("ts_hl", [npc, l, 2 * C], bf16, kind="ExternalInput")
    demT = nc.dram_tensor("demT_aug", [DEM + 1, npc], f32, kind="ExternalInput")
    w1a = nc.dram_tensor("w1_aug", [DEM + 1, H1], f32, kind="ExternalInput")
    w2a = nc.dram_tensor("w2_aug", [H1 + 1, DEMF], f32, kind="ExternalInput")
    tri = nc.dram_tensor("tri", [P, P], bf16, kind="ExternalInput")
    ltri = nc.dram_tensor("ltri", [P, P], bf16, kind="ExternalInput")
    rcol = nc.dram_tensor("rcol", [P, nchunks], f32, kind="ExternalInput")
    out = nc.dram_tensor("out", [npc, l, COUT], f32, kind="ExternalOutput")

    with tile.TileContext(nc) as tc, ExitStack() as ctx:
        const = ctx.enter_context(tc.tile_pool(name="const", bufs=1))
        T_t = const.tile([P, P], bf16)
        nc.gpsimd.dma_start(T_t[:], tri.ap())
        U_t = const.tile([P, P], bf16)
        nc.gpsimd.dma_start(U_t[:], ltri.ap())
        rcol_t = const.tile([P, nchunks], f32)
        nc.gpsimd.dma_start(rcol_t[:], rcol.ap())
        demb = const.tile([P, npc * DEMF], f32)
        _emit_mlp_and_demb(nc, tc, mybir, npc, demT, w1a, w2a, demb)

        accp = ctx.enter_context(tc.tile_pool(name="acc", bufs=npc, space="PSUM"))
        inp = ctx.enter_context(tc.tile_pool(name="xin", bufs=inbufs))
        outp = ctx.enter_context(tc.tile_pool(name="outb", bufs=outbufs))

        acc = [accp.tile([P, C], f32, name="acc", tag="acc") for _ in range(npc)]
        ts_ap = ts.ap()
        out_ap = out.ap()

        # Emission order = per-engine queue order (engines are in-order).
        # Batch each stage across all patients so the PE never has to sit
        # behind one patient's readout while other patients' matmuls are
        # ready, and stays dense enough for the HAM clock to hold 2.4 GHz.
        for gi in range(ng):
            l0 = gi * g * P
            xins = []
            for n in range(npc):
                xin = inp.tile([P, g * 2 * C], bf16, name="xin", tag="xin")
                nc.sync.dma_start(
                    xin[:].rearrange("p (g k) -> p g k", k=2 * C),
                    ts_ap[n, l0:l0 + g * P, :].rearrange("(g p) c -> p g c", p=P))
                xins.append(xin)
            for j in range(g):
                ch = gi * g + j
                for n in range(npc):
                    hi = xins[n][:, j * 2 * C:j * 2 * C + C]
                    lo = xins[n][:, j * 2 * C + C:(j + 1) * 2 * C]
                    nc.tensor.matmul(acc[n][:], T_t[:], hi,
                                     start=(ch == 0), stop=False,
                                     skip_group_check=True)
                    nc.tensor.matmul(acc[n][:], T_t[:], lo,
                                     start=False, stop=(ch == nchunks - 1),
                                     skip_group_check=True)
                outcs = []
                for n in range(npc):
                    outc = outp.tile([P, COUT], f32, name="outc", tag="outc")
                    outcs.append(outc)
                    # readout: relu(csum)/l == relu(csum * (1/l)); split the
                    # work across DVE and the mostly-idle ACT engine
                    if n % 2 == 0:
                        nc.vector.tensor_scalar(
                            outc[:, 0:C],
                            acc[n][:], 0.0, rcol_t[:, ch:ch + 1],
                            OP.max, OP.mult)
                    else:
                        nc.scalar.activation(
                            outc[:, 0:C],
                            acc[n][:], AF.Relu, scale=rcol_t[:, ch:ch + 1])
                    nc.vector.tensor_copy(
                        outc[:, C:COUT], demb[:, n * DEMF:(n + 1) * DEMF])
                if ch != nchunks - 1:
                    for n in range(npc):
                        hi = xins[n][:, j * 2 * C:j * 2 * C + C]
                        lo = xins[n][:, j * 2 * C + C:(j + 1) * 2 * C]
                        nc.tensor.matmul(acc[n][:], U_t[:], hi,
                                         start=False, stop=False,
                                         skip_group_check=True)
                        nc.tensor.matmul(acc[n][:], U_t[:], lo,
                                         start=False, stop=False,
                                         skip_group_check=True)
                for n in range(npc):
                    nc.scalar.dma_start(
                        out_ap[n, ch * P:(ch + 1) * P, :], outcs[n][:])

    nc.compile()
    return nc


def _build_general(npc=NPC, nchunks=NCHUNKS, g=G):
    """Correct for any input: [x | ind] fp32r matmuls + approx reciprocal."""
    import concourse.mybir as mybir
    import concourse.tile as tile
    from concourse import bacc
    from contextlib import ExitStack

    f32 = mybir.dt.float32
    f32r = mybir.dt.float32r
    AF = mybir.ActivationFunctionType
    OP = mybir.AluOpType

    l = nchunks * P
    ng = nchunks // g

    nc = bacc.Bacc("TRN2", target_bir_lowering=False, debug=False,
                   num_devices=NCORES)
    ts = nc.dram_tensor("ts", [npc, l, C], f32r, kind="ExternalInput")
    demT = nc.dram_tensor("demT_aug", [DEM + 1, npc], f32, kind="ExternalInput")
    w1a = nc.dram_tensor("w1_aug", [DEM + 1, H1], f32, kind="ExternalInput")
    w2a = nc.dram_tensor("w2_aug", [H1 + 1, DEMF], f32, kind="ExternalInput")
    tri = nc.dram_tensor("tri", [P, P], f32r, kind="ExternalInput")
    ltri = nc.dram_tensor("ltri", [P, P], f32r, kind="ExternalInput")
    out = nc.dram_tensor("out", [npc, l, COUT], f32, kind="ExternalOutput")

    with tile.TileContext(nc) as tc, ExitStack() as ctx:
        const = ctx.enter_context(tc.tile_pool(name="const", bufs=1))
        T_t = const.tile([P, P], f32r)
        nc.sync.dma_start(T_t[:], tri.ap())
        U_t = const.tile([P, P], f32r)
        nc.sync.dma_start(U_t[:], ltri.ap())
        demb = const.tile([P, npc * DEMF], f32)
        _emit_mlp_and_demb(nc, tc, mybir, npc, demT, w1a, w2a, demb)

        accp = ctx.enter_context(tc.tile_pool(name="acc", bufs=npc, space="PSUM"))
        inp = ctx.enter_context(tc.tile_pool(name="xin", bufs=4))
        rcpp = ctx.enter_context(tc.tile_pool(name="rcp", bufs=6))
        outp = ctx.enter_context(tc.tile_pool(name="outb", bufs=4))

        acc = [accp.tile([P, 2 * C], f32, name="acc", tag="acc") for _ in range(npc)]
        ts_ap = ts.ap()
        out_ap = out.ap()

        for gi in range(ng):
            l0 = gi * g * P
            for n in range(npc):
                xin = inp.tile([P, g * 2 * C], f32r)
                xv = xin[:].rearrange("p (g k) -> p g k", k=2 * C)
                nc.sync.dma_start(
                    xv[:, :, 0:C],
                    ts_ap[n, l0:l0 + g * P, :].rearrange("(g p) c -> p g c", p=P))
                nc.vector.tensor_scalar(
                    xv[:, :, C:2 * C], xv[:, :, 0:C], 0.0, EPS,
                    OP.not_equal, OP.add)
                outt = outp.tile([P, g * COUT], f32)
                for j in range(g):
                    ch = gi * g + j
                    rhs = xin[:, j * 2 * C:(j + 1) * 2 * C]
                    nc.tensor.matmul(acc[n][:], T_t[:], rhs,
                                     start=(ch == 0), stop=(ch == nchunks - 1),
                                     skip_group_check=True)
                    rcp_t = rcpp.tile([P, C], f32)
                    nc.vector.reciprocal_approx_fast(
                        out=rcp_t[:], in_=acc[n][:, C:2 * C])
                    nc.vector.scalar_tensor_tensor(
                        out=outt[:, j * COUT:j * COUT + C],
                        in0=acc[n][:, 0:C], scalar=0.0, in1=rcp_t[:],
                        op0=OP.max, op1=OP.mult)
                    nc.scalar.activation(
                        outt[:, j * COUT + C:(j + 1) * COUT],
                        demb[:, n * DEMF:(n + 1) * DEMF], AF.Copy)
                    if ch != nchunks - 1:
                        nc.tensor.matmul(acc[n][:], U_t[:], rhs,
                                         start=False, stop=False,
                                         skip_group_check=True)
                nc.sync.dma_start(
                    out_ap[n, l0:l0 + g * P, :].rearrange("(g p) c -> p g c", p=P),
                    outt[:].rearrange("p (g c) -> p g c", c=COUT))

    nc.compile()
    return nc


def _mlp_inputs(dem, w1, b1, w2, b2):
    n = dem.shape[0]
    demT_aug = np.concatenate([dem.T, np.ones((1, n), np.float32)], 0)
    w1_aug = np.concatenate([w1, b1[None, :]], 0)
    w2_aug = np.concatenate([w2, b2[None, :]], 0)
    return demT_aug, w1_aug, w2_aug


def _tri_np(dtype):
    k = np.arange(P)
    tri = (k[:, None] <= k[None, :]).astype(dtype)
    ltri = (k[:, None] > k[None, :]).astype(dtype)
    return tri, ltri


def _host_inputs_general(dem, timesteps, w1, b1, w2, b2, npc=NPC, l=L):
    demT_aug, w1_aug, w2_aug = _mlp_inputs(dem, w1, b1, w2, b2)
    tri, ltri = _tri_np(np.float32)
    ncores = dem.shape[0] // npc
    in_maps = []
    for c in range(ncores):
        in_maps.append({
            "ts": np.ascontiguousarray(timesteps[c * npc:(c + 1) * npc, :l]),
            "demT_aug": np.ascontiguousarray(demT_aug[:, c * npc:(c + 1) * npc]),
            "w1_aug": w1_aug, "w2_aug": w2_aug, "tri": tri, "ltri": ltri,
        })
    return in_maps


def _host_inputs_fast(dem, timesteps, w1, b1, w2, b2, npc=NPC, l=L,
                      nchunks=NCHUNKS):
    import ml_dtypes

    demT_aug, w1_aug, w2_aug = _mlp_inputs(dem, w1, b1, w2, b2)
    tri, ltri = _tri_np(ml_dtypes.bfloat16)
    n = timesteps.shape[0]
    hi = timesteps.astype(ml_dtypes.bfloat16)
    lo = (timesteps - hi.astype(np.float32)).astype(ml_dtypes.bfloat16)
    ts_hl = np.concatenate([hi, lo], axis=-1)  # [n, l, 2C] bf16
    li = np.arange(l, dtype=np.float64) + 1.0
    rcol = (1.0 / li).astype(np.float32).reshape(nchunks, P).T  # [P, nchunks]
    rcol = np.ascontiguousarray(rcol)
    ncores = n // npc
    in_maps = []
    for c in range(ncores):
        in_maps.append({
            "ts_hl": np.ascontiguousarray(ts_hl[c * npc:(c + 1) * npc, :l]),
            "demT_aug": np.ascontiguousarray(demT_aug[:, c * npc:(c + 1) * npc]),
            "w1_aug": w1_aug, "w2_aug": w2_aug, "tri": tri, "ltri": ltri,
            "rcol": rcol,
        })
    return in_maps


def kernel(dem, timesteps, w1, b1, w2, b2):
    global LAST_EXEC_NS, LAST_MODE
    from concourse.bass_utils import run_bass_kernel_spmd

    dem = np.asarray(dem, np.float32)
    timesteps = np.asarray(timesteps, np.float32)
    w1 = np.asarray(w1, np.float32)
    b1 = np.asarray(b1, np.float32)
    w2 = np.asarray(w2, np.float32)
    b2 = np.asarray(b2, np.float32)

    if TRACE:
        _register_ntff_hook()

    mode = "general" if (timesteps == 0).any() else "fast"
    LAST_MODE = mode
    if mode not in _COMPILED:
        _COMPILED[mode] = (_build_fast() if mode == "fast"
                           else _build_general())
    nc = _COMPILED[mode]
    if mode == "fast":
        in_maps = _host_inputs_fast(dem, timesteps, w1, b1, w2, b2)
    else:
        in_maps = _host_inputs_general(dem, timesteps, w1, b1, w2, b2)
    res = run_bass_kernel_spmd(nc, in_maps, list(range(NCORES)), trace=TRACE)
    LAST_EXEC_NS = res.exec_time_ns
    outs = [res.results[c]["out"] for c in range(NCORES)]
    return np.concatenate(outs, axis=0)


# revision 25
# speedup vs baseline: 1.1297x; 1.1297x over previous
"""PatientMeanEncoder Trainium2 kernel.

Computes, for full inputs (dem [64,10], timesteps [64,2048,256], MLP weights):
    d = relu(relu(dem@w1+b1)@w2+b2)                      # [64,20]
    x = concat([timesteps, broadcast(d)], -1)            # [64,2048,276]
    out = relu(cumsum(x,1) / max(cumsum(x!=0,1), 1))     # [64,2048,276]

Sharding: pure data parallel over 8 NeuronCores, 8 patients per core
(timesteps/out sliced on N; tiny MLP weights replicated; each core runs
its own patients' MLP rows).

Core algorithm (both modes): per patient, the causal cumulative sums
live in a PSUM bank. For each 128-row L-chunk, a matmul with the
inclusive upper-triangular T (T[k,m]=1 for k<=m) accumulates the
in-chunk prefix on top of the carry already in the bank; after the
readout, a strictly-lower U' (U'[k,m]=1 for k>m) matmul tops the bank
up to the full-chunk column total, which is exactly the carry the next
chunk needs. Each element passes the PE twice; everything stays on-chip.

Two compiled variants, dispatched on the host by scanning the input:

- fast: valid when timesteps contains no exact zeros. Then the nonzero
  count for the timesteps channels is deterministically l+1, so the
  whole count cumsum disappears; the readout is a single tensor_scalar
  (relu via op0=max, then multiply by a host-precomputed per-partition
  1/(l+1) column). x is split on the host into bf16 hi+lo (x ~= hi+lo
  to ~2^-17): same DMA bytes as f32, but the matmuls run at full bf16
  PE rate instead of the ~3x-slower fp32 path, with hi and lo
  accumulating into the same PSUM columns.

- general: correct for any input. ind = (x != 0) + 1e-35 is computed on
  DVE and rides in the same [x | ind] fp32r moving operand (FD=512);
  the epsilon keeps count>0 everywhere (where the true count is 0 the
  cumsum is exactly 0, so out = 0 * huge = 0), removing any clamp op.
  Readout is a custom-DVE approximate reciprocal (~51 ULP) plus one
  scalar_tensor_tensor.

The dem block of the output is d broadcast along L (exactly d: for
those channels avg == d whether d is zero or not): a per-patient SBUF
tile written once via a DRAM-bounce broadcast DMA, copied into each
output tile by the otherwise-idle ACT engine.
"""

import os
import sys
import types
import numpy as np

# Problem constants (hardcoded per contract; kernel.py is self-contained).
N, L, C, DEM = 64, 2048, 256, 10
H1, DEMF = 40, 20
NCORES = 8
NPC = N // NCORES            # patients per core
P = 128                      # partitions = rows per L-chunk
NCHUNKS = L // P             # 16
G = 2                        # L-chunks per DMA group
COUT = C + DEMF              # 276
EPS = 1e-35

_COMPILED = {}
LAST_EXEC_NS = None
LAST_MODE = None
TRACE = os.environ.get("PME_TRACE", "1") == "1"


def _register_ntff_hook():
    """This image's antenv lacks axon_hooks; synthesize it so
    run_bass_kernel_spmd(trace=True) can capture NTFF profiles.
    Degrades silently (trace is skipped) if anything is missing."""
    try:
        import antenv.axon_hooks  # noqa: F401
        return
    except Exception:
        pass
    try:
        from trn_agent_boot.trn_boot import _ntff_profile_via_ctypes

        hook = _ntff_profile_via_ctypes("/opt/axon/libaxon_pjrt.so")
        mod = types.ModuleType("antenv.axon_hooks")
        mod.get_axon_ntff_profile_hook = lambda: hook
        mod.set_axon_ntff_profile_hook = lambda h: None
        sys.modules["antenv.axon_hooks"] = mod
        import antenv

        antenv.axon_hooks = mod
    except Exception:
        pass


def _emit_mlp_and_demb(nc, tc, mybir, npc, demT, w1a, w2a, demb):
    """dem_fc MLP (biases folded via augmented ones row/column) +
    per-patient broadcast tiles of d along the partition dim."""
    f32 = mybir.dt.float32
    AF = mybir.ActivationFunctionType
    with tc.tile_pool(name="mlps", bufs=1) as mlps, \
         tc.tile_pool(name="mlpp", bufs=2, space="PSUM") as mlpp:
        demT_t = mlps.tile([DEM + 1, npc], f32)
        nc.gpsimd.dma_start(demT_t[:], demT.ap())
        w1_t = mlps.tile([DEM + 1, H1], f32)
        nc.gpsimd.dma_start(w1_t[:], w1a.ap())
        w2_t = mlps.tile([H1 + 1, DEMF], f32)
        nc.gpsimd.dma_start(w2_t[:], w2a.ap())
        p1 = mlpp.tile([H1, npc], f32)
        nc.tensor.matmul(p1[:], w1_t[:], demT_t[:], start=True, stop=True)
        h1 = mlps.tile([H1 + 1, npc], f32)
        nc.vector.memset(h1[:], 1.0)  # row H1 stays 1.0 (bias input)
        nc.scalar.activation(h1[0:H1, :], p1[:], AF.Relu)
        p2 = mlpp.tile([npc, DEMF], f32)
        nc.tensor.matmul(p2[:], h1[:], w2_t[:], start=True, stop=True)
        d_t = mlps.tile([npc, DEMF], f32)
        nc.scalar.activation(d_t[:], p2[:], AF.Relu)
        # SBUF APs can't partition-broadcast in DMA; bounce via DRAM.
        dscratch = nc.dram_tensor("dscratch", [npc, DEMF], f32)
        nc.gpsimd.dma_start(dscratch.ap(), d_t[:])
        for pi in range(npc):
            nc.gpsimd.dma_start(
                demb[:, pi * DEMF:(pi + 1) * DEMF],
                dscratch.ap()[pi, :].partition_broadcast(P))


def _build_fast(npc=NPC, nchunks=NCHUNKS, g=4, inbufs=16, outbufs=12):
    """No-exact-zeros variant: count == l+1, x as bf16 hi+lo."""
    import concourse.mybir as mybir
    import concourse.tile as tile
    from concourse import bacc
    from contextlib import ExitStack

    f32 = mybir.dt.float32
    bf16 = mybir.dt.bfloat16
    AF = mybir.ActivationFunctionType
    OP = mybir.AluOpType

    l = nchunks * P
    ng = nchunks // g

    nc = bacc.Bacc("TRN2", target_bir_lowering=False, debug=False,
                   num_devices=NCORES)
    # host-packed [hi | lo] bf16 per row: [npc, l, 2*C] bf16
    ts = nc.dram_tensor("ts_hl", [npc, l, 2 * C], bf16, kind="ExternalInput")
    demT = nc.dram_tensor("demT_aug", [DEM + 1, npc], f32, kind="ExternalInput")
    w1a = nc.dram_tensor("w1_aug", [DEM + 1, H1], f32, kind="ExternalInput")
    w2a = nc.dram_tensor("w2_aug", [H1 + 1, DEMF], f32, kind="ExternalInput")
    tri = nc.dram_tensor("tri", [P, P], bf16, kind="ExternalInput")
    ltri = nc.dram_tensor("ltri", [P, P], bf16, kind="ExternalInput")
    rcol = nc.dram_tensor("rcol", [P, nchunks], f32, kind="ExternalInput")
    out = nc.dram_tensor("out", [npc, l, COUT], f32, kind="ExternalOutput")

    with tile.TileContext(nc) as tc, ExitStack() as ctx:
        const = ctx.enter_context(tc.tile_pool(name="const", bufs=1))
        T_t = const.tile([P, P], bf16)
        nc.gpsimd.dma_start(T_t[:], tri.ap())
        U_t = const.tile([P, P], bf16)
        nc.gpsimd.dma_start(U_t[:], ltri.ap())
        rcol_t = const.tile([P, nchunks], f32)
        nc.gpsimd.dma_start(rcol_t[:], rcol.ap())
        demb = const.tile([P, npc * DEMF], f32)
        _emit_mlp_and_demb(nc, tc, mybir, npc, demT, w1a, w2a, demb)

        accp = ctx.enter_context(tc.tile_pool(name="acc", bufs=npc, space="PSUM"))
        inp = ctx.enter_context(tc.tile_pool(name="xin", bufs=inbufs))
        outp = ctx.enter_context(tc.tile_pool(name="outb", bufs=outbufs))

        acc = [accp.tile([P, C], f32, name="acc", tag="acc") for _ in range(npc)]
        ts_ap = ts.ap()
        out_ap = out.ap()

        # Emission order = per-engine queue order (engines are in-order).
        # Batch each stage across all patients so the PE never has to sit
        # behind one patient's readout while other patients' matmuls are
        # ready, and stays dense enough for the HAM clock to hold 2.4 GHz.
        for gi in range(ng):
            l0 = gi * g * P
            xins = []
            outts = []
            for n in range(npc):
                xin = inp.tile([P, g * 2 * C], bf16, name="xin", tag="xin")
                nc.sync.dma_start(
                    xin[:].rearrange("p (g k) -> p g k", k=2 * C),
                    ts_ap[n, l0:l0 + g * P, :].rearrange("(g p) c -> p g c", p=P))
                xins.append(xin)
                outts.append(outp.tile([P, g * COUT], f32, name="outt",
                                       tag="outt"))
            for j in range(g):
                ch = gi * g + j
                for n in range(npc):
                    hi = xins[n][:, j * 2 * C:j * 2 * C + C]
                    lo = xins[n][:, j * 2 * C + C:(j + 1) * 2 * C]
                    nc.tensor.matmul(acc[n][:], T_t[:], hi,
                                     start=(ch == 0), stop=False,
                                     skip_group_check=True)
                    nc.tensor.matmul(acc[n][:], T_t[:], lo,
                                     start=False, stop=(ch == nchunks - 1),
                                     skip_group_check=True)
                for n in range(npc):
                    # readout: relu(csum)/l == relu(csum * (1/l)); split the
                    # work across DVE and the mostly-idle ACT engine
                    if n % 2 == 0:
                        nc.vector.tensor_scalar(
                            outts[n][:, j * COUT:j * COUT + C],
                            acc[n][:], 0.0, rcol_t[:, ch:ch + 1],
                            OP.max, OP.mult)
                    else:
                        nc.scalar.activation(
                            outts[n][:, j * COUT:j * COUT + C],
                            acc[n][:], AF.Relu, scale=rcol_t[:, ch:ch + 1])
                if ch != nchunks - 1:
                    for n in range(npc):
                        hi = xins[n][:, j * 2 * C:j * 2 * C + C]
                        lo = xins[n][:, j * 2 * C + C:(j + 1) * 2 * C]
                        nc.tensor.matmul(acc[n][:], U_t[:], hi,
                                         start=False, stop=False,
                                         skip_group_check=True)
                        nc.tensor.matmul(acc[n][:], U_t[:], lo,
                                         start=False, stop=False,
                                         skip_group_check=True)
            for n in range(npc):
                nc.scalar.activation(
                    outts[n][:].rearrange("p (g c) -> p g c", c=COUT)[:, :, C:COUT],
                    demb[:, None, n * DEMF:(n + 1) * DEMF].broadcast_to(
                        [P, g, DEMF]),
                    AF.Copy)
                nc.scalar.dma_start(
                    out_ap[n, l0:l0 + g * P, :].rearrange("(g p) c -> p g c", p=P),
                    outts[n][:].rearrange("p (g c) -> p g c", c=COUT))

    nc.compile()
    return nc


def _build_general(npc=NPC, nchunks=NCHUNKS, g=G):
    """Correct for any input: [x | ind] fp32r matmuls + approx reciprocal."""
    import concourse.mybir as mybir
    import concourse.tile as tile
    from concourse import bacc
    from contextlib import ExitStack

    f32 = mybir.dt.float32
    f32r = mybir.dt.float32r
    AF = mybir.ActivationFunctionType
    OP = mybir.AluOpType

    l = nchunks * P
    ng = nchunks // g

    nc = bacc.Bacc("TRN2", target_bir_lowering=False, debug=False,
                   num_devices=NCORES)
    ts = nc.dram_tensor("ts", [npc, l, C], f32r, kind="ExternalInput")
    demT = nc.dram_tensor("demT_aug", [DEM + 1, npc], f32, kind="ExternalInput")
    w1a = nc.dram_tensor("w1_aug", [DEM + 1, H1], f32, kind="ExternalInput")
    w2a = nc.dram_tensor("w2_aug", [H1 + 1, DEMF], f32, kind="ExternalInput")
    tri = nc.dram_tensor("tri", [P, P], f32r, kind="ExternalInput")
    ltri = nc.dram_tensor("ltri", [P, P], f32r, kind="ExternalInput")
    out = nc.dram_tensor("out", [npc, l, COUT], f32, kind="ExternalOutput")

    with tile.TileContext(nc) as tc, ExitStack() as ctx:
        const = ctx.enter_context(tc.tile_pool(name="const", bufs=1))
        T_t = const.tile([P, P], f32r)
        nc.sync.dma_start(T_t[:], tri.ap())
        U_t = const.tile([P, P], f32r)
        nc.sync.dma_start(U_t[:], ltri.ap())
        demb = const.tile([P, npc * DEMF], f32)
        _emit_mlp_and_demb(nc, tc, mybir, npc, demT, w1a, w2a, demb)

        accp = ctx.enter_context(tc.tile_pool(name="acc", bufs=npc, space="PSUM"))
        inp = ctx.enter_context(tc.tile_pool(name="xin", bufs=4))
        rcpp = ctx.enter_context(tc.tile_pool(name="rcp", bufs=6))
        outp = ctx.enter_context(tc.tile_pool(name="outb", bufs=4))

        acc = [accp.tile([P, 2 * C], f32, name="acc", tag="acc") for _ in range(npc)]
        ts_ap = ts.ap()
        out_ap = out.ap()

        for gi in range(ng):
            l0 = gi * g * P
            for n in range(npc):
                xin = inp.tile([P, g * 2 * C], f32r)
                xv = xin[:].rearrange("p (g k) -> p g k", k=2 * C)
                nc.sync.dma_start(
                    xv[:, :, 0:C],
                    ts_ap[n, l0:l0 + g * P, :].rearrange("(g p) c -> p g c", p=P))
                nc.vector.tensor_scalar(
                    xv[:, :, C:2 * C], xv[:, :, 0:C], 0.0, EPS,
                    OP.not_equal, OP.add)
                outt = outp.tile([P, g * COUT], f32)
                for j in range(g):
                    ch = gi * g + j
                    rhs = xin[:, j * 2 * C:(j + 1) * 2 * C]
                    nc.tensor.matmul(acc[n][:], T_t[:], rhs,
                                     start=(ch == 0), stop=(ch == nchunks - 1),
                                     skip_group_check=True)
                    rcp_t = rcpp.tile([P, C], f32)
                    nc.vector.reciprocal_approx_fast(
                        out=rcp_t[:], in_=acc[n][:, C:2 * C])
                    nc.vector.scalar_tensor_tensor(
                        out=outt[:, j * COUT:j * COUT + C],
                        in0=acc[n][:, 0:C], scalar=0.0, in1=rcp_t[:],
                        op0=OP.max, op1=OP.mult)
                    nc.scalar.activation(
                        outt[:, j * COUT + C:(j + 1) * COUT],
                        demb[:, n * DEMF:(n + 1) * DEMF], AF.Copy)
                    if ch != nchunks - 1:
                        nc.tensor.matmul(acc[n][:], U_t[:], rhs,
                                         start=False, stop=False,
                                         skip_group_check=True)
                nc.sync.dma_start(
                    out_ap[n, l0:l0 + g * P, :].rearrange("(g p) c -> p g c", p=P),
                    outt[:].rearrange("p (g c) -> p g c", c=COUT))

    nc.compile()
    return nc


def _mlp_inputs(dem, w1, b1, w2, b2):
    n = dem.shape[0]
    demT_aug = np.concatenate([dem.T, np.ones((1, n), np.float32)], 0)
    w1_aug = np.concatenate([w1, b1[None, :]], 0)
    w2_aug = np.concatenate([w2, b2[None, :]], 0)
    return demT_aug, w1_aug, w2_aug


def _tri_np(dtype):
    k = np.arange(P)
    tri = (k[:, None] <= k[None, :]).astype(dtype)
    ltri = (k[:, None] > k[None, :]).astype(dtype)
    return tri, ltri


def _host_inputs_general(dem, timesteps, w1, b1, w2, b2, npc=NPC, l=L):
    demT_aug, w1_aug, w2_aug = _mlp_inputs(dem, w1, b1, w2, b2)
    tri, ltri = _tri_np(np.float32)
    ncores = dem.shape[0] // npc
    in_maps = []
    for c in range(ncores):
        in_maps.append({
            "ts": np.ascontiguousarray(timesteps[c * npc:(c + 1) * npc, :l]),
            "demT_aug": np.ascontiguousarray(demT_aug[:, c * npc:(c + 1) * npc]),
            "w1_aug": w1_aug, "w2_aug": w2_aug, "tri": tri, "ltri": ltri,
        })
    return in_maps


def _host_inputs_fast(dem, timesteps, w1, b1, w2, b2, npc=NPC, l=L,
                      nchunks=NCHUNKS):
    import ml_dtypes

    demT_aug, w1_aug, w2_aug = _mlp_inputs(dem, w1, b1, w2, b2)
    tri, ltri = _tri_np(ml_dtypes.bfloat16)
    n = timesteps.shape[0]
    hi = timesteps.astype(ml_dtypes.bfloat16)
    lo = (timesteps - hi.astype(np.float32)).astype(ml_dtypes.bfloat16)
    ts_hl = np.concatenate([hi, lo], axis=-1)  # [n, l, 2C] bf16
    li = np.arange(l, dtype=np.float64) + 1.0
    rcol = (1.0 / li).astype(np.float32).reshape(nchunks, P).T  # [P, nchunks]
    rcol = np.ascontiguousarray(rcol)
    ncores = n // npc
    in_maps = []
    for c in range(ncores):
        in_maps.append({
            "ts_hl": np.ascontiguousarray(ts_hl[c * npc:(c + 1) * npc, :l]),
            "demT_aug": np.ascontiguousarray(demT_aug[:, c * npc:(c + 1) * npc]),
            "w1_aug": w1_aug, "w2_aug": w2_aug, "tri": tri, "ltri": ltri,
            "rcol": rcol,
        })
    return in_maps


def _sanity_ok(out, dem, timesteps, w1, b1, w2, b2, rng):
    """Cheap host spot-checks to catch gross device/transport corruption
    (observed once as a transient): first-row identity, random causal-mean
    samples, and dem-block constancy. Loose tolerances — only wholesale
    wrongness trips this."""
    d = np.maximum(dem @ w1 + b1, 0.0)
    d = np.maximum(d @ w2 + b2, 0.0)
    # dem block == d broadcast along L (sampled rows)
    if not np.allclose(out[:, ::509, C:], d[:, None, :], atol=1e-2):
        return False
    # l=0: avg == x itself
    if not np.allclose(out[:, 0, :C], np.maximum(timesteps[:, 0, :], 0.0),
                       atol=1e-2, rtol=1e-2):
        return False
    # random causal-mean spot checks
    for _ in range(12):
        n = int(rng.integers(0, out.shape[0]))
        li = int(rng.integers(1, out.shape[1]))
        exp = np.maximum(timesteps[n, :li + 1, :].mean(0), 0.0)
        if not np.allclose(out[n, li, :C], exp, atol=1e-2, rtol=1e-2):
            return False
    return True


def kernel(dem, timesteps, w1, b1, w2, b2):
    global LAST_EXEC_NS, LAST_MODE
    from concourse.bass_utils import run_bass_kernel_spmd

    dem = np.asarray(dem, np.float32)
    timesteps = np.asarray(timesteps, np.float32)
    w1 = np.asarray(w1, np.float32)
    b1 = np.asarray(b1, np.float32)
    w2 = np.asarray(w2, np.float32)
    b2 = np.asarray(b2, np.float32)

    if TRACE:
        _register_ntff_hook()

    mode = "general" if (timesteps == 0).any() else "fast"
    rng = np.random.default_rng(12345)

    def run(m):
        global LAST_EXEC_NS, LAST_MODE
        if m not in _COMPILED:
            _COMPILED[m] = (_build_fast() if m == "fast"
                            else _build_general())
        if m == "fast":
            in_maps = _host_inputs_fast(dem, timesteps, w1, b1, w2, b2)
        else:
            in_maps = _host_inputs_general(dem, timesteps, w1, b1, w2, b2)
        res = run_bass_kernel_spmd(_COMPILED[m], in_maps,
                                   list(range(NCORES)), trace=TRACE)
        LAST_EXEC_NS = res.exec_time_ns
        LAST_MODE = m
        return np.concatenate([res.results[c]["out"] for c in range(NCORES)],
                              axis=0)

    # fast mode only valid without exact zeros; sanity-checked with retry,
    # then general-path fallback (independent program) as a last resort.
    attempts = (["fast", "fast", "general"] if mode == "fast"
                else ["general", "general"])
    out = None
    for m in attempts[:-1]:
        out = run(m)
        if _sanity_ok(out, dem, timesteps, w1, b1, w2, b2, rng):
            return out
    return run(attempts[-1])


# revision 28
# speedup vs baseline: 1.2506x; 1.1070x over previous
"""PatientMeanEncoder Trainium2 kernel.

Computes, for full inputs (dem [64,10], timesteps [64,2048,256], MLP weights):
    d = relu(relu(dem@w1+b1)@w2+b2)                      # [64,20]
    x = concat([timesteps, broadcast(d)], -1)            # [64,2048,276]
    out = relu(cumsum(x,1) / max(cumsum(x!=0,1), 1))     # [64,2048,276]

Sharding: pure data parallel over 8 NeuronCores, 8 patients per core
(timesteps/out sliced on N; tiny MLP weights replicated; each core runs
its own patients' MLP rows).

Core algorithm (both modes): per patient, the causal cumulative sums
live in a PSUM bank. For each 128-row L-chunk, a matmul with the
inclusive upper-triangular T (T[k,m]=1 for k<=m) accumulates the
in-chunk prefix on top of the carry already in the bank; after the
readout, a strictly-lower U' (U'[k,m]=1 for k>m) matmul tops the bank
up to the full-chunk column total, which is exactly the carry the next
chunk needs. Each element passes the PE twice; everything stays on-chip.

Two compiled variants, dispatched on the host by scanning the input:

- fast: valid when timesteps contains no exact zeros. Then the nonzero
  count for the timesteps channels is deterministically l+1, so the
  whole count cumsum disappears; the readout is a single tensor_scalar
  (relu via op0=max, then multiply by a host-precomputed per-partition
  1/(l+1) column). x is split on the host into bf16 hi+lo (x ~= hi+lo
  to ~2^-17): same DMA bytes as f32, but the matmuls run at full bf16
  PE rate instead of the ~3x-slower fp32 path, with hi and lo
  accumulating into the same PSUM columns.

- general: correct for any input. ind = (x != 0) + 1e-35 is computed on
  DVE and rides in the same [x | ind] fp32r moving operand (FD=512);
  the epsilon keeps count>0 everywhere (where the true count is 0 the
  cumsum is exactly 0, so out = 0 * huge = 0), removing any clamp op.
  Readout is a custom-DVE approximate reciprocal (~51 ULP) plus one
  scalar_tensor_tensor.

The dem block of the output is d broadcast along L (exactly d: for
those channels avg == d whether d is zero or not): a per-patient SBUF
tile written once via a DRAM-bounce broadcast DMA, copied into each
output tile by the otherwise-idle ACT engine.
"""

import os
import sys
import types
import numpy as np

# Problem constants (hardcoded per contract; kernel.py is self-contained).
N, L, C, DEM = 64, 2048, 256, 10
H1, DEMF = 40, 20
NCORES = 8
NPC = N // NCORES            # patients per core
P = 128                      # partitions = rows per L-chunk
NCHUNKS = L // P             # 16
G = 2                        # L-chunks per DMA group
COUT = C + DEMF              # 276
EPS = 1e-35

_COMPILED = {}
LAST_EXEC_NS = None
LAST_MODE = None
TRACE = os.environ.get("PME_TRACE", "1") == "1"


def _register_ntff_hook():
    """This image's antenv lacks axon_hooks; synthesize it so
    run_bass_kernel_spmd(trace=True) can capture NTFF profiles.
    Degrades silently (trace is skipped) if anything is missing."""
    try:
        import antenv.axon_hooks  # noqa: F401
        return
    except Exception:
        pass
    try:
        from trn_agent_boot.trn_boot import _ntff_profile_via_ctypes

        hook = _ntff_profile_via_ctypes("/opt/axon/libaxon_pjrt.so")
        mod = types.ModuleType("antenv.axon_hooks")
        mod.get_axon_ntff_profile_hook = lambda: hook
        mod.set_axon_ntff_profile_hook = lambda h: None
        sys.modules["antenv.axon_hooks"] = mod
        import antenv

        antenv.axon_hooks = mod
    except Exception:
        pass


def _emit_mlp_and_demb(nc, tc, mybir, npc, demT, w1a, w2a, demb):
    """dem_fc MLP (biases folded via augmented ones row/column) +
    per-patient broadcast tiles of d along the partition dim."""
    f32 = mybir.dt.float32
    AF = mybir.ActivationFunctionType
    with tc.tile_pool(name="mlps", bufs=1) as mlps, \
         tc.tile_pool(name="mlpp", bufs=2, space="PSUM") as mlpp:
        demT_t = mlps.tile([DEM + 1, npc], f32)
        nc.gpsimd.dma_start(demT_t[:], demT.ap())
        w1_t = mlps.tile([DEM + 1, H1], f32)
        nc.gpsimd.dma_start(w1_t[:], w1a.ap())
        w2_t = mlps.tile([H1 + 1, DEMF], f32)
        nc.gpsimd.dma_start(w2_t[:], w2a.ap())
        p1 = mlpp.tile([H1, npc], f32)
        nc.tensor.matmul(p1[:], w1_t[:], demT_t[:], start=True, stop=True)
        h1 = mlps.tile([H1 + 1, npc], f32)
        nc.vector.memset(h1[:], 1.0)  # row H1 stays 1.0 (bias input)
        nc.scalar.activation(h1[0:H1, :], p1[:], AF.Relu)
        p2 = mlpp.tile([npc, DEMF], f32)
        nc.tensor.matmul(p2[:], h1[:], w2_t[:], start=True, stop=True)
        d_t = mlps.tile([npc, DEMF], f32)
        nc.scalar.activation(d_t[:], p2[:], AF.Relu)
        # SBUF APs can't partition-broadcast in DMA; bounce via DRAM.
        dscratch = nc.dram_tensor("dscratch", [npc, DEMF], f32)
        nc.gpsimd.dma_start(dscratch.ap(), d_t[:])
        for pi in range(npc):
            nc.gpsimd.dma_start(
                demb[:, pi * DEMF:(pi + 1) * DEMF],
                dscratch.ap()[pi, :].partition_broadcast(P))


def _build_fast(npc=NPC, nchunks=NCHUNKS, g=2, inbufs=10, outbufs=10):
    """No-exact-zeros variant: count == l+1, x as bf16 hi+lo, and TWO
    patients packed per PSUM bank so every matmul/readout/DMA covers a
    patient pair (FD=512 bf16 matmuls; half the instructions and sync)."""
    import concourse.mybir as mybir
    import concourse.tile as tile
    from concourse import bacc
    from contextlib import ExitStack

    f32 = mybir.dt.float32
    bf16 = mybir.dt.bfloat16
    AF = mybir.ActivationFunctionType
    OP = mybir.AluOpType

    l = nchunks * P
    ng = nchunks // g
    nq = npc // 2  # patient pairs

    nc = bacc.Bacc("TRN2", target_bir_lowering=False, debug=False,
                   num_devices=NCORES)
    # host-packed per pair q, per row l: [hi(2q) | hi(2q+1) | lo(2q) | lo(2q+1)]
    ts = nc.dram_tensor("ts_hl2", [nq, l, 4 * C], bf16, kind="ExternalInput")
    demT = nc.dram_tensor("demT_aug", [DEM + 1, npc], f32, kind="ExternalInput")
    w1a = nc.dram_tensor("w1_aug", [DEM + 1, H1], f32, kind="ExternalInput")
    w2a = nc.dram_tensor("w2_aug", [H1 + 1, DEMF], f32, kind="ExternalInput")
    tri = nc.dram_tensor("tri", [P, P], bf16, kind="ExternalInput")
    ltri = nc.dram_tensor("ltri", [P, P], bf16, kind="ExternalInput")
    rcol = nc.dram_tensor("rcol", [P, nchunks], f32, kind="ExternalInput")
    out = nc.dram_tensor("out", [npc, l, COUT], f32, kind="ExternalOutput")

    with tile.TileContext(nc) as tc, ExitStack() as ctx:
        const = ctx.enter_context(tc.tile_pool(name="const", bufs=1))
        T_t = const.tile([P, P], bf16)
        nc.gpsimd.dma_start(T_t[:], tri.ap())
        U_t = const.tile([P, P], bf16)
        nc.gpsimd.dma_start(U_t[:], ltri.ap())
        rcol_t = const.tile([P, nchunks], f32)
        nc.gpsimd.dma_start(rcol_t[:], rcol.ap())
        demb = const.tile([P, npc * DEMF], f32)
        _emit_mlp_and_demb(nc, tc, mybir, npc, demT, w1a, w2a, demb)

        accp = ctx.enter_context(tc.tile_pool(name="acc", bufs=nq, space="PSUM"))
        inp = ctx.enter_context(tc.tile_pool(name="xin", bufs=inbufs))
        outp = ctx.enter_context(tc.tile_pool(name="outb", bufs=outbufs))

        # one [128, 512] PSUM bank holds a patient pair's accumulators
        acc = [accp.tile([P, 2 * C], f32, name="acc", tag="acc")
               for _ in range(nq)]
        ts_ap = ts.ap()
        out_ap = out.ap()

        # Emission order = per-engine queue order (engines are in-order).
        # Batch each stage across all pairs so the PE never has to sit
        # behind one pair's readout while other pairs' matmuls are ready.
        for gi in range(ng):
            l0 = gi * g * P
            xins = []
            outts = []
            for q in range(nq):
                xin = inp.tile([P, g * 4 * C], bf16, name="xin", tag="xin")
                nc.sync.dma_start(
                    xin[:].rearrange("p (g k) -> p g k", k=4 * C),
                    ts_ap[q, l0:l0 + g * P, :].rearrange("(g p) c -> p g c", p=P))
                xins.append(xin)
                outts.append(outp.tile([P, g * 2 * COUT], f32, name="outt",
                                       tag="outt"))
            for j in range(g):
                ch = gi * g + j
                for q in range(nq):
                    hi2 = xins[q][:, j * 4 * C:j * 4 * C + 2 * C]
                    lo2 = xins[q][:, j * 4 * C + 2 * C:(j + 1) * 4 * C]
                    nc.tensor.matmul(acc[q][:], T_t[:], hi2,
                                     start=(ch == 0), stop=False,
                                     skip_group_check=True)
                    nc.tensor.matmul(acc[q][:], T_t[:], lo2,
                                     start=False, stop=(ch == nchunks - 1),
                                     skip_group_check=True)
                for q in range(nq):
                    # readout: relu(csum)/l == relu(csum * (1/l)); one op per
                    # pair, split across DVE and the mostly-idle ACT engine
                    ov = outts[q][:].rearrange(
                        "p (g u c) -> p g u c", u=2, c=COUT)[:, j, :, 0:C]
                    if q % 2 == 0:
                        nc.vector.tensor_scalar(
                            ov, acc[q][:].rearrange("p (u c) -> p u c", c=C),
                            0.0, rcol_t[:, ch:ch + 1], OP.max, OP.mult)
                    else:
                        nc.scalar.activation(
                            ov, acc[q][:].rearrange("p (u c) -> p u c", c=C),
                            AF.Relu, scale=rcol_t[:, ch:ch + 1])
                if ch != nchunks - 1:
                    for q in range(nq):
                        hi2 = xins[q][:, j * 4 * C:j * 4 * C + 2 * C]
                        lo2 = xins[q][:, j * 4 * C + 2 * C:(j + 1) * 4 * C]
                        nc.tensor.matmul(acc[q][:], U_t[:], hi2,
                                         start=False, stop=False,
                                         skip_group_check=True)
                        nc.tensor.matmul(acc[q][:], U_t[:], lo2,
                                         start=False, stop=False,
                                         skip_group_check=True)
            for q in range(nq):
                nc.scalar.activation(
                    outts[q][:].rearrange(
                        "p (g u c) -> p g u c", u=2, c=COUT)[:, :, :, C:COUT],
                    demb[:, None, 2 * q * DEMF:(2 * q + 2) * DEMF]
                    .broadcast_to([P, g, 2 * DEMF])
                    .rearrange("p g (u c) -> p g u c", c=DEMF),
                    AF.Copy)
                for u in range(2):
                    nc.scalar.dma_start(
                        out_ap[2 * q + u, l0:l0 + g * P, :]
                        .rearrange("(g p) c -> p g c", p=P),
                        outts[q][:].rearrange(
                            "p (g u c) -> p g u c", u=2, c=COUT)[:, :, u, :])

    nc.compile()
    return nc


def _build_general(npc=NPC, nchunks=NCHUNKS, g=G):
    """Correct for any input: [x | ind] fp32r matmuls + approx reciprocal."""
    import concourse.mybir as mybir
    import concourse.tile as tile
    from concourse import bacc
    from contextlib import ExitStack

    f32 = mybir.dt.float32
    f32r = mybir.dt.float32r
    AF = mybir.ActivationFunctionType
    OP = mybir.AluOpType

    l = nchunks * P
    ng = nchunks // g

    nc = bacc.Bacc("TRN2", target_bir_lowering=False, debug=False,
                   num_devices=NCORES)
    ts = nc.dram_tensor("ts", [npc, l, C], f32r, kind="ExternalInput")
    demT = nc.dram_tensor("demT_aug", [DEM + 1, npc], f32, kind="ExternalInput")
    w1a = nc.dram_tensor("w1_aug", [DEM + 1, H1], f32, kind="ExternalInput")
    w2a = nc.dram_tensor("w2_aug", [H1 + 1, DEMF], f32, kind="ExternalInput")
    tri = nc.dram_tensor("tri", [P, P], f32r, kind="ExternalInput")
    ltri = nc.dram_tensor("ltri", [P, P], f32r, kind="ExternalInput")
    out = nc.dram_tensor("out", [npc, l, COUT], f32, kind="ExternalOutput")

    with tile.TileContext(nc) as tc, ExitStack() as ctx:
        const = ctx.enter_context(tc.tile_pool(name="const", bufs=1))
        T_t = const.tile([P, P], f32r)
        nc.sync.dma_start(T_t[:], tri.ap())
        U_t = const.tile([P, P], f32r)
        nc.sync.dma_start(U_t[:], ltri.ap())
        demb = const.tile([P, npc * DEMF], f32)
        _emit_mlp_and_demb(nc, tc, mybir, npc, demT, w1a, w2a, demb)

        accp = ctx.enter_context(tc.tile_pool(name="acc", bufs=npc, space="PSUM"))
        inp = ctx.enter_context(tc.tile_pool(name="xin", bufs=4))
        rcpp = ctx.enter_context(tc.tile_pool(name="rcp", bufs=6))
        outp = ctx.enter_context(tc.tile_pool(name="outb", bufs=4))

        acc = [accp.tile([P, 2 * C], f32, name="acc", tag="acc") for _ in range(npc)]
        ts_ap = ts.ap()
        out_ap = out.ap()

        for gi in range(ng):
            l0 = gi * g * P
            for n in range(npc):
                xin = inp.tile([P, g * 2 * C], f32r)
                xv = xin[:].rearrange("p (g k) -> p g k", k=2 * C)
                nc.sync.dma_start(
                    xv[:, :, 0:C],
                    ts_ap[n, l0:l0 + g * P, :].rearrange("(g p) c -> p g c", p=P))
                nc.vector.tensor_scalar(
                    xv[:, :, C:2 * C], xv[:, :, 0:C], 0.0, EPS,
                    OP.not_equal, OP.add)
                outt = outp.tile([P, g * COUT], f32)
                for j in range(g):
                    ch = gi * g + j
                    rhs = xin[:, j * 2 * C:(j + 1) * 2 * C]
                    nc.tensor.matmul(acc[n][:], T_t[:], rhs,
                                     start=(ch == 0), stop=(ch == nchunks - 1),
                                     skip_group_check=True)
                    rcp_t = rcpp.tile([P, C], f32)
                    nc.vector.reciprocal_approx_fast(
                        out=rcp_t[:], in_=acc[n][:, C:2 * C])
                    nc.vector.scalar_tensor_tensor(
                        out=outt[:, j * COUT:j * COUT + C],
                        in0=acc[n][:, 0:C], scalar=0.0, in1=rcp_t[:],
                        op0=OP.max, op1=OP.mult)
                    nc.scalar.activation(
                        outt[:, j * COUT + C:(j + 1) * COUT],
                        demb[:, n * DEMF:(n + 1) * DEMF], AF.Copy)
                    if ch != nchunks - 1:
                        nc.tensor.matmul(acc[n][:], U_t[:], rhs,
                                         start=False, stop=False,
                                         skip_group_check=True)
                nc.sync.dma_start(
                    out_ap[n, l0:l0 + g * P, :].rearrange("(g p) c -> p g c", p=P),
                    outt[:].rearrange("p (g c) -> p g c", c=COUT))

    nc.compile()
    return nc


def _mlp_inputs(dem, w1, b1, w2, b2):
    n = dem.shape[0]
    demT_aug = np.concatenate([dem.T, np.ones((1, n), np.float32)], 0)
    w1_aug = np.concatenate([w1, b1[None, :]], 0)
    w2_aug = np.concatenate([w2, b2[None, :]], 0)
    return demT_aug, w1_aug, w2_aug


def _tri_np(dtype):
    k = np.arange(P)
    tri = (k[:, None] <= k[None, :]).astype(dtype)
    ltri = (k[:, None] > k[None, :]).astype(dtype)
    return tri, ltri


def _host_inputs_general(dem, timesteps, w1, b1, w2, b2, npc=NPC, l=L):
    demT_aug, w1_aug, w2_aug = _mlp_inputs(dem, w1, b1, w2, b2)
    tri, ltri = _tri_np(np.float32)
    ncores = dem.shape[0] // npc
    in_maps = []
    for c in range(ncores):
        in_maps.append({
            "ts": np.ascontiguousarray(timesteps[c * npc:(c + 1) * npc, :l]),
            "demT_aug": np.ascontiguousarray(demT_aug[:, c * npc:(c + 1) * npc]),
            "w1_aug": w1_aug, "w2_aug": w2_aug, "tri": tri, "ltri": ltri,
        })
    return in_maps


def _host_inputs_fast(dem, timesteps, w1, b1, w2, b2, npc=NPC, l=L,
                      nchunks=NCHUNKS):
    import ml_dtypes

    demT_aug, w1_aug, w2_aug = _mlp_inputs(dem, w1, b1, w2, b2)
    tri, ltri = _tri_np(ml_dtypes.bfloat16)
    n = timesteps.shape[0]
    hi = timesteps.astype(ml_dtypes.bfloat16)
    lo = (timesteps - hi.astype(np.float32)).astype(ml_dtypes.bfloat16)
    # pair layout: [hi(2q) | hi(2q+1) | lo(2q) | lo(2q+1)] per row
    ts_hl2 = np.concatenate(
        [hi[0::2], hi[1::2], lo[0::2], lo[1::2]], axis=-1)  # [n/2, l, 4C]
    li = np.arange(l, dtype=np.float64) + 1.0
    rcol = (1.0 / li).astype(np.float32).reshape(nchunks, P).T  # [P, nchunks]
    rcol = np.ascontiguousarray(rcol)
    ncores = n // npc
    nqc = npc // 2
    in_maps = []
    for c in range(ncores):
        in_maps.append({
            "ts_hl2": np.ascontiguousarray(ts_hl2[c * nqc:(c + 1) * nqc, :l]),
            "demT_aug": np.ascontiguousarray(demT_aug[:, c * npc:(c + 1) * npc]),
            "w1_aug": w1_aug, "w2_aug": w2_aug, "tri": tri, "ltri": ltri,
            "rcol": rcol,
        })
    return in_maps


def _sanity_ok(out, dem, timesteps, w1, b1, w2, b2, rng):
    """Cheap host spot-checks to catch gross device/transport corruption
    (observed once as a transient): first-row identity, random causal-mean
    samples, and dem-block constancy. Loose tolerances — only wholesale
    wrongness trips this."""
    d = np.maximum(dem @ w1 + b1, 0.0)
    d = np.maximum(d @ w2 + b2, 0.0)
    # dem block == d broadcast along L (sampled rows)
    if not np.allclose(out[:, ::509, C:], d[:, None, :], atol=1e-2):
        return False
    # l=0: avg == x itself
    if not np.allclose(out[:, 0, :C], np.maximum(timesteps[:, 0, :], 0.0),
                       atol=1e-2, rtol=1e-2):
        return False
    # random causal-mean spot checks
    for _ in range(12):
        n = int(rng.integers(0, out.shape[0]))
        li = int(rng.integers(1, out.shape[1]))
        exp = np.maximum(timesteps[n, :li + 1, :].mean(0), 0.0)
        if not np.allclose(out[n, li, :C], exp, atol=1e-2, rtol=1e-2):
            return False
    return True


def kernel(dem, timesteps, w1, b1, w2, b2):
    global LAST_EXEC_NS, LAST_MODE
    from concourse.bass_utils import run_bass_kernel_spmd

    dem = np.asarray(dem, np.float32)
    timesteps = np.asarray(timesteps, np.float32)
    w1 = np.asarray(w1, np.float32)
    b1 = np.asarray(b1, np.float32)
    w2 = np.asarray(w2, np.float32)
    b2 = np.asarray(b2, np.float32)

    if TRACE:
        _register_ntff_hook()

    mode = "general" if (timesteps == 0).any() else "fast"
    rng = np.random.default_rng(12345)

    def run(m):
        global LAST_EXEC_NS, LAST_MODE
        if m not in _COMPILED:
            _COMPILED[m] = (_build_fast() if m == "fast"
                            else _build_general())
        if m == "fast":
            in_maps = _host_inputs_fast(dem, timesteps, w1, b1, w2, b2)
        else:
            in_maps = _host_inputs_general(dem, timesteps, w1, b1, w2, b2)
        res = run_bass_kernel_spmd(_COMPILED[m], in_maps,
                                   list(range(NCORES)), trace=TRACE)
        LAST_EXEC_NS = res.exec_time_ns
        LAST_MODE = m
        return np.concatenate([res.results[c]["out"] for c in range(NCORES)],
                              axis=0)

    # fast mode only valid without exact zeros; sanity-checked with retry,
    # then general-path fallback (independent program) as a last resort.
    attempts = (["fast", "fast", "general"] if mode == "fast"
                else ["general", "general"])
    out = None
    for m in attempts[:-1]:
        out = run(m)
        if _sanity_ok(out, dem, timesteps, w1, b1, w2, b2, rng):
            return out
    return run(attempts[-1])
